# revision 68
# baseline (speedup 1.0000x reference)
"""Bahdanau-attention Bass kernel for 8 TRN2 NeuronCores (data-parallel over batch).

Shapes (hardcoded): B=128, S=1024, EH2=1024, DH=512, A=512.
Returns (context [B, EH2] f32, attn_weights [B, S] f32) matching the reference.

Per core (16 batch rows, no cross-core communication), everything heavy on PE:
  - Host ships encoder_outputs twice in bf16: transposed [e, s] chunks (proj
    matmul contracts over partitions) and natural [s, e] chunks (context
    matmul contracts over s). Host prep is free w.r.t. HW exec time.
  - projT[a, s] = sum_e W_enc[e, a] * encT[e, s]  (64 MMs/row, bf16)
  - energyT = tanh(projT + dec_projT[:, row] + b_attnT), fused bias on ACT,
    one tile per a-chunk so scores only wait on the chunk they read.
  - scores[1, s] = sum_a v[a] * energyT[a, s] (8 MMs, M=1)
    (+ optional additive log-mask as an extra accumulation when the mask
    isn't all-ones)
  - exp on ACT with accum_out -> sum; reciprocal on DVE.
  - expT[s%128, s-chunk] via 8 K=1 matmuls (row -> partitions).
  - ctx[1, e] = sum_s expT-weighted enc (16 MMs, M=1, accumulate over s).
  - The expT + ctx matmuls for row r-1 are emitted inside row r's PE stream
    (after proj, around scores) so the in-order PE never waits on the
    softmax chain.
"""

import os

import numpy as np
import ml_dtypes

B, S, E, DH, A = 128, 1024, 1024, 512, 512
NCORES = 8

LAST_EXEC_NS = None

_NC_CACHE = {}


def _build_nc(rows, has_mask):
    import concourse.tile as tile
    from concourse import bacc, mybir

    f32 = mybir.dt.float32
    bf16 = mybir.dt.bfloat16
    Tanh = mybir.ActivationFunctionType.Tanh
    Exp = mybir.ActivationFunctionType.Exp
    Ident = mybir.ActivationFunctionType.Identity

    nc = bacc.Bacc(
        "TRN2", target_bir_lowering=False, debug=False, num_devices=NCORES
    )

    encT_d = nc.declare_dram_parameter("encT", [rows, 128, 8, S], bf16, isOutput=False)
    encN_d = nc.declare_dram_parameter("encN", [3, 128, 8, E], bf16, isOutput=False)
    w_d = nc.declare_dram_parameter("w", [128, 8, A], bf16, isOutput=False)
    dpT_d = nc.declare_dram_parameter("dpT", [128, 4, rows], f32, isOutput=False)
    vcols_d = nc.declare_dram_parameter("vcols", [128, 4], bf16, isOutput=False)
    ident_d = nc.declare_dram_parameter("ident", [128, 128], f32, isOutput=False)
    if has_mask:
        maskf_d = nc.declare_dram_parameter("maskf", [rows, S], bf16, isOutput=False)
    out_d = nc.declare_dram_parameter("out", [rows, E + S], f32, isOutput=True)
    attn_bounce = nc.dram_tensor("attn_bounce", [rows, S], bf16)

    with tile.TileContext(nc) as tc:
        with (
            tc.tile_pool(name="singles", bufs=1) as singles,
            tc.tile_pool(name="encT_pool", bufs=5) as encT_pool,
            tc.tile_pool(name="energy_pool", bufs=2) as energy_pool,
            tc.tile_pool(name="small", bufs=3) as small,
            tc.tile_pool(name="outp", bufs=3) as outp,
            tc.tile_pool(name="mmps", bufs=4, space="PSUM") as mmps,
            tc.tile_pool(name="vecps", bufs=2, space="PSUM") as vecps,
        ):
            dpT_sb = singles.tile([128, 4, rows], f32)
            nc.scalar.dma_start(out=dpT_sb[:], in_=dpT_d[:])
            w_sb = [
                singles.tile([128, A], bf16, name=f"w_c{k}") for k in range(8)
            ]
            for k in range(8):
                nc.scalar.dma_start(out=w_sb[k][:], in_=w_d[:, k, :])
            vcols_sb = singles.tile([128, 4], bf16)
            nc.gpsimd.dma_start(out=vcols_sb[:], in_=vcols_d[:])
            ident_sb = singles.tile([128, 128], f32)
            nc.gpsimd.dma_start(out=ident_sb[:], in_=ident_d[:])
            if has_mask:
                one11 = singles.tile([1, 1], bf16)
                nc.vector.memset(one11, 1.0)
            one_f = singles.tile([1, 1], f32)
            nc.vector.memset(one_f, 1.0)

            import concourse.bass as bass

            def emit_bcast(st):
                # attnbf -> DRAM bounce -> stride-0 partition broadcast
                r = st["r"]
                nc.scalar.dma_start(
                    out=attn_bounce[r : r + 1, :], in_=st["attnbf"][:]
                )
                expb_sb = small.tile([128, S], bf16, name="expb_sb", bufs=3)
                st["expb_sb"] = expb_sb
                brow = attn_bounce[r : r + 1, :]
                bcast_ap = bass.AP(
                    tensor=brow.tensor,
                    offset=brow.offset,
                    ap=[[0, 128], brow.ap[-1]],
                )
                nc.scalar.dma_start(out=expb_sb[:], in_=bcast_ap)

            def emit_ctx_dve(st):
                # ctxT[e%128, chunk] = sum_s encT * attn on DVE (muls +
                # reduces 0..nd-1) and ACT (reduces nd..7), software-pipelined
                nd = st["nd"]
                encT_t, expb_sb = st["encT_t"], st["expb_sb"]
                ct_dve = small.tile([128, nd], mybir.dt.float32, name="ct_dve", tag="ctd", bufs=5)
                ct_act = small.tile(
                    [128, 8 - nd], mybir.dt.float32, name="ct_act", tag="ctc", bufs=5
                )
                st["ct_dve"], st["ct_act"] = ct_dve, ct_act
                scr = [
                    small.tile(
                        [128, S], bf16, name=f"scr{i}", tag=f"scr{i}",
                        bufs=2 if i >= 6 else 1,
                    )
                    for i in range(8)
                ]

                def cmul(c):
                    nc.vector.tensor_mul(scr[c][:], encT_t[c][:], expb_sb[:])

                def cred(c):
                    nc.vector.reduce_sum(
                        out=ct_dve[:, c : c + 1], in_=scr[c][:], axis=mybir.AxisListType.X
                    )

                cmul(0)
                cmul(1)
                for c in range(2, 8):
                    cmul(c)
                    if c - 2 < nd:
                        cred(c - 2)
                st["scr"] = scr

            def emit_ctx_act(st):
                # ACT reductions for chunks nd..7 — emitted after the NEXT
                # row's tanh so they never block it in the ACT FIFO
                nd = st["nd"]
                for c in range(nd, 8):
                    nc.scalar.activation(
                        out=st["scr"][c][:],
                        in_=st["scr"][c][:],
                        func=Ident,
                        accum_out=st["ct_act"][:, c - nd : c - nd + 1],
                    )

            def emit_pe_expT(st):
                # expm row -> partitions via K=1 matmuls (PE)
                st["expT_ps"] = mmps.tile(
                    [128, 8], mybir.dt.float32, tag="mm", name="expT_ps"
                )
                for c in range(8):
                    nc.tensor.matmul(
                        st["expT_ps"][:, c : c + 1],
                        lhsT=st["expm"][0:1, c * 128 : (c + 1) * 128],
                        rhs=one_f[:],
                        start=True,
                        stop=True,
                    )

            def emit_pe_ctx(st):
                r, encN_t = st["r"], st["encN_t"]
                expT_sb = small.tile([128, 8], bf16, name="expT_sb")
                nc.vector.tensor_copy(out=expT_sb[:], in_=st["expT_ps"][:])
                ctx_ps = vecps.tile([1, E], mybir.dt.float32, tag="v", name="ctx_ps")
                for eh in range(2):
                    for c in range(8):
                        nc.tensor.matmul(
                            ctx_ps[0:1, eh * 512 : (eh + 1) * 512],
                            lhsT=expT_sb[:, c : c + 1],
                            rhs=encN_t[c][:, eh * 512 : (eh + 1) * 512],
                            start=(c == 0),
                            stop=(c == 7),
                        )
                outctx = outp.tile([1, E], mybir.dt.float32, tag="outctx", bufs=2)
                nc.scalar.activation(
                    out=outctx[:], in_=ctx_ps[0:1, :], func=Ident, scale=st["recip"]
                )
                nc.gpsimd.dma_start(out=out_d[r : r + 1, 0:E], in_=outctx[:])

            def emit_ctx_out(st):
                # transposes on PE (deferred), copies + DMA out
                r, nd = st["r"], st["nd"]
                na = 8 - nd
                ps_d = vecps.tile([nd, 128], mybir.dt.float32, tag="v", name="ps_d")
                nc.tensor.transpose(ps_d[:], st["ct_dve"][:], ident_sb[:])
                ps_a = vecps.tile([na, 128], mybir.dt.float32, tag="v", name="ps_a")
                nc.tensor.transpose(ps_a[:], st["ct_act"][:], ident_sb[:])
                outc_d = outp.tile([nd, 1, 128], mybir.dt.float32, tag="outcd", bufs=2)
                nc.scalar.copy(out=outc_d[:, 0, :], in_=ps_d[:])
                outc_a = outp.tile([na, 1, 128], mybir.dt.float32, tag="outca", bufs=2)
                nc.scalar.copy(out=outc_a[:, 0, :], in_=ps_a[:])
                nc.gpsimd.dma_start(
                    out=out_d[r : r + 1, 0 : nd * 128].rearrange(
                        "o (j f) -> j o f", j=nd
                    ),
                    in_=outc_d[:],
                )
                nc.gpsimd.dma_start(
                    out=out_d[r : r + 1, nd * 128 : 1024].rearrange(
                        "o (j f) -> j o f", j=na
                    ),
                    in_=outc_a[:],
                )

            # hoisted prefetch: the first two rows' chunk DMAs go out before
            # any result-gated DMA exists, split across both input rings
            prefetched = []
            for rr in range(min(2, rows)):
                tiles = [
                    encT_pool.tile([128, S], bf16, name=f"encc{c}", tag=f"encc{c}")
                    for c in range(8)
                ]
                for c in range(8):
                    eng = nc.scalar if c % 2 == 1 else nc.sync
                    eng.dma_start(out=tiles[c][:], in_=encT_d[rr, :, c, :])
                prefetched.append(tiles)

            pend_out = []
            pend_pe = []
            encN_pending = {}
            for r in range(rows):
                if r < len(prefetched):
                    encT_t = prefetched[r]
                else:
                    encT_t = [
                        encT_pool.tile([128, S], bf16, name=f"encc{c}", tag=f"encc{c}")
                        for c in range(8)
                    ]
                    for c in range(8):
                        nc.sync.dma_start(out=encT_t[c][:], in_=encT_d[r, :, c, :])
                pe_ctx_row = r >= rows - 3
                # prefetch encN (natural layout for the PE-ctx tail rows) two
                # rows early so its DMA issue load spreads out
                rpre = r + 2
                if rows - 3 <= rpre < rows:
                    tiles = [
                        encT_pool.tile([128, E], bf16, name=f"encn{c}", tag=f"encn{c}", bufs=2)
                        for c in range(8)
                    ]
                    for c in range(8):
                        nc.sync.dma_start(
                            out=tiles[c][:], in_=encN_d[rpre - (rows - 3), :, c, :]
                        )
                    encN_pending[rpre] = tiles
                encN_t = encN_pending.pop(r, None)
                if pe_ctx_row and encN_t is None:
                    encN_t = [
                        encT_pool.tile([128, E], bf16, name=f"encn{c}", tag=f"encn{c}", bufs=2)
                        for c in range(8)
                    ]
                    for c in range(8):
                        nc.sync.dma_start(
                            out=encN_t[c][:], in_=encN_d[r - (rows - 3), :, c, :]
                        )
                if has_mask:
                    maskrow = small.tile([1, S], bf16, name="maskrow", bufs=2)
                    nc.sync.dma_start(out=maskrow[:], in_=maskf_d[r : r + 1, :])

                # projT -> tanh -> energyT (bf16), a on partitions
                energyT = []
                for m in range(4):
                    et = energy_pool.tile(
                        [128, S], bf16, name=f"energyT{m}", tag=f"e{m}"
                    )
                    energyT.append(et)
                    for n in range(2):
                        mm = mmps.tile([128, 512], mybir.dt.float32, tag="mm")
                        for k in range(8):
                            nc.tensor.matmul(
                                mm[:],
                                lhsT=w_sb[k][:, m * 128 : (m + 1) * 128],
                                rhs=encT_t[k][:, n * 512 : (n + 1) * 512],
                                start=(k == 0),
                                stop=(k == 7),
                            )
                        nc.scalar.activation(
                            out=et[:, n * 512 : (n + 1) * 512],
                            in_=mm[:],
                            func=Tanh,
                            bias=dpT_sb[:, m, r : r + 1],
                            scale=1.0,
                        )

                # ACT reductions of the previous row (after this row's tanh)
                if pend_out:
                    last = pend_out[-1]
                    if not last.get("act_done") and "scr" in last:
                        emit_ctx_act(last)
                        last["act_done"] = True

                # deferred PE work for earlier rows
                if pend_pe:
                    emit_pe_expT(pend_pe[0])
                if len(pend_out) >= 3:
                    emit_ctx_out(pend_out.pop(0))

                # scores flat [1, S]
                scores_ps = vecps.tile([1, S], mybir.dt.float32, tag="v")
                for n in range(2):
                    for m in range(4):
                        nc.tensor.matmul(
                            scores_ps[0:1, n * 512 : (n + 1) * 512],
                            lhsT=vcols_sb[:, m : m + 1],
                            rhs=energyT[m][:, n * 512 : (n + 1) * 512],
                            start=(m == 0),
                            stop=(m == 3) and not has_mask,
                        )
                    if has_mask:
                        nc.tensor.matmul(
                            scores_ps[0:1, n * 512 : (n + 1) * 512],
                            lhsT=one11[:],
                            rhs=maskrow[0:1, n * 512 : (n + 1) * 512],
                            start=False,
                            stop=True,
                        )

                # softmax smalls
                expm = small.tile([1, S], mybir.dt.float32, name="expm", bufs=4)
                srtile = small.tile([1, 8], mybir.dt.float32)
                ssum = srtile[0:1, 0:1]
                recip = srtile[0:1, 1:2]
                nc.scalar.activation(
                    out=expm[:], in_=scores_ps[0:1, :], func=Exp, accum_out=ssum
                )
                nc.vector.reciprocal(out=recip, in_=ssum)

                # attn output (f32) + normalized bf16 copy for the broadcast
                outa = outp.tile([1, S], mybir.dt.float32, tag="outa", bufs=2)
                nc.scalar.activation(out=outa[:], in_=expm[:], func=Ident, scale=recip)
                nc.gpsimd.dma_start(out=out_d[r : r + 1, E : E + S], in_=outa[:])
                attnbf = small.tile([1, S], bf16, name="attnbf")
                nc.scalar.activation(
                    out=attnbf[:], in_=expm[:], func=Ident, scale=recip
                )

                if pend_pe:
                    emit_pe_ctx(pend_pe.pop(0))

                st = {
                    "r": r,
                    "encT_t": encT_t,
                    "attnbf": attnbf,
                    "expm": expm,
                    "recip": recip,
                    "encN_t": encN_t,
                    "nd": 6,
                }
                if pe_ctx_row:
                    pend_pe.append(st)
                else:
                    emit_bcast(st)
                    emit_ctx_dve(st)
                    pend_out.append(st)

            for st in pend_out:
                if not st.get("act_done") and "scr" in st:
                    emit_ctx_act(st)
                    st["act_done"] = True
            while pend_out:
                emit_ctx_out(pend_out.pop(0))
            while pend_pe:
                s0 = pend_pe.pop(0)
                emit_pe_expT(s0)
                emit_pe_ctx(s0)

    nc.compile()
    return nc


def _prep_inputs(encoder_outputs, decoder_hidden, src_mask, W_attn, b_attn, v):
    bf16 = ml_dtypes.bfloat16
    b = encoder_outputs.shape[0]
    rows = b // NCORES

    enc_bf = np.asarray(encoder_outputs, dtype=np.float32).astype(bf16)
    # encT_a[b, p, c, s] = enc[b, s, c*128+p]
    encT_a = np.ascontiguousarray(enc_bf.reshape(b, S, 8, 128).transpose(0, 3, 2, 1))
    # natural layout for the last 3 rows of each core: [b, p, c, e] = enc[b, c*128+p, e]
    encN_a = np.ascontiguousarray(enc_bf.reshape(b, 8, 128, E).transpose(0, 2, 1, 3))

    W = np.asarray(W_attn, dtype=np.float32)
    w_a = np.ascontiguousarray(W[:E].astype(bf16).reshape(8, 128, A).transpose(1, 0, 2))
    W2 = W[E:]
    b_np = np.asarray(b_attn, dtype=np.float32)
    vcols = np.ascontiguousarray(
        np.asarray(v, dtype=np.float32).astype(bf16).reshape(4, 128).T
    )
    ident = np.eye(128, dtype=np.float32)

    dh = np.asarray(decoder_hidden, dtype=np.float32)
    mask_np = np.asarray(src_mask)
    has_mask = bool((mask_np == 0).any())
    # additive log-mask: 0 where kept, -1e4 where masked (exp -> exactly 0)
    maskf = np.where(mask_np != 0, 0.0, -1e4).astype(bf16)

    in_maps = []
    for i in range(NCORES):
        sl = slice(i * rows, (i + 1) * rows)
        dh_sh = dh[sl]  # [rows, DH]
        # dec_proj + b on host (exact f32), transposed to [128, 4, rows]
        dp = (dh_sh @ W2 + b_np).astype(np.float32)  # [rows, A]
        dpT_a = np.ascontiguousarray(
            dp.T.reshape(4, 128, rows).transpose(1, 0, 2)
        )
        m = {
            "encT": encT_a[sl],
            "encN": encN_a[i * rows + rows - 3 : (i + 1) * rows],
            "w": w_a,
            "dpT": dpT_a,
            "vcols": vcols,
            "ident": ident,
        }
        if has_mask:
            m["maskf"] = np.ascontiguousarray(maskf[sl])
        in_maps.append(m)
    return in_maps, rows, has_mask


def kernel(encoder_outputs, decoder_hidden, src_mask, W_attn, b_attn, v):
    global LAST_EXEC_NS
    from concourse.bass_utils import run_bass_kernel_spmd

    in_maps, rows, has_mask = _prep_inputs(
        encoder_outputs, decoder_hidden, src_mask, W_attn, b_attn, v
    )

    key = (rows, has_mask)
    if key not in _NC_CACHE:
        _NC_CACHE[key] = _build_nc(rows, has_mask)
    nc = _NC_CACHE[key]

    trace = os.environ.get("KERNEL_TRACE", "0") == "1"
    res = run_bass_kernel_spmd(nc, in_maps, core_ids=list(range(NCORES)), trace=trace)
    LAST_EXEC_NS = res.exec_time_ns

    ctx = np.concatenate([r["out"][:, :E] for r in res.results], axis=0)
    attn = np.concatenate([r["out"][:, E:] for r in res.results], axis=0)
    return ctx.astype(np.float32), attn.astype(np.float32)


# revision 69
# speedup vs baseline: 1.0251x; 1.0251x over previous
"""Bahdanau-attention Bass kernel for 8 TRN2 NeuronCores (data-parallel over batch).

Shapes (hardcoded): B=128, S=1024, EH2=1024, DH=512, A=512.
Returns (context [B, EH2] f32, attn_weights [B, S] f32) matching the reference.

Per core (16 batch rows, no cross-core communication), everything heavy on PE:
  - Host ships encoder_outputs twice in bf16: transposed [e, s] chunks (proj
    matmul contracts over partitions) and natural [s, e] chunks (context
    matmul contracts over s). Host prep is free w.r.t. HW exec time.
  - projT[a, s] = sum_e W_enc[e, a] * encT[e, s]  (64 MMs/row, bf16)
  - energyT = tanh(projT + dec_projT[:, row] + b_attnT), fused bias on ACT,
    one tile per a-chunk so scores only wait on the chunk they read.
  - scores[1, s] = sum_a v[a] * energyT[a, s] (8 MMs, M=1)
    (+ optional additive log-mask as an extra accumulation when the mask
    isn't all-ones)
  - exp on ACT with accum_out -> sum; reciprocal on DVE.
  - expT[s%128, s-chunk] via 8 K=1 matmuls (row -> partitions).
  - ctx[1, e] = sum_s expT-weighted enc (16 MMs, M=1, accumulate over s).
  - The expT + ctx matmuls for row r-1 are emitted inside row r's PE stream
    (after proj, around scores) so the in-order PE never waits on the
    softmax chain.
"""

import os

import numpy as np
import ml_dtypes

B, S, E, DH, A = 128, 1024, 1024, 512, 512
NCORES = 8

LAST_EXEC_NS = None

_NC_CACHE = {}


def _build_nc(rows, has_mask):
    import concourse.tile as tile
    from concourse import bacc, mybir

    f32 = mybir.dt.float32
    bf16 = mybir.dt.bfloat16
    Tanh = mybir.ActivationFunctionType.Tanh
    Exp = mybir.ActivationFunctionType.Exp
    Ident = mybir.ActivationFunctionType.Identity

    nc = bacc.Bacc(
        "TRN2", target_bir_lowering=False, debug=False, num_devices=NCORES
    )

    encT_d = nc.declare_dram_parameter("encT", [rows, 128, 8, S], bf16, isOutput=False)
    encN_d = nc.declare_dram_parameter("encN", [3, 128, 8, E], bf16, isOutput=False)
    w_d = nc.declare_dram_parameter("w", [128, 8, A], bf16, isOutput=False)
    dpT_d = nc.declare_dram_parameter("dpT", [128, 4, rows], f32, isOutput=False)
    vcols_d = nc.declare_dram_parameter("vcols", [128, 4], bf16, isOutput=False)
    ident_d = nc.declare_dram_parameter("ident", [128, 128], f32, isOutput=False)
    if has_mask:
        maskf_d = nc.declare_dram_parameter("maskf", [rows, S], bf16, isOutput=False)
    out_d = nc.declare_dram_parameter("out", [rows, E + S], f32, isOutput=True)
    attn_bounce = nc.dram_tensor("attn_bounce", [rows, S], bf16)

    with tile.TileContext(nc) as tc:
        with (
            tc.tile_pool(name="singles", bufs=1) as singles,
            tc.tile_pool(name="encT_pool", bufs=5) as encT_pool,
            tc.tile_pool(name="energy_pool", bufs=2) as energy_pool,
            tc.tile_pool(name="small", bufs=3) as small,
            tc.tile_pool(name="outp", bufs=3) as outp,
            tc.tile_pool(name="mmps", bufs=4, space="PSUM") as mmps,
            tc.tile_pool(name="vecps", bufs=2, space="PSUM") as vecps,
        ):
            dpT_sb = singles.tile([128, 4, rows], f32)
            nc.scalar.dma_start(out=dpT_sb[:], in_=dpT_d[:])
            w_sb = [
                singles.tile([128, A], bf16, name=f"w_c{k}") for k in range(8)
            ]
            for k in range(8):
                nc.scalar.dma_start(out=w_sb[k][:], in_=w_d[:, k, :])
            vcols_sb = singles.tile([128, 4], bf16)
            nc.gpsimd.dma_start(out=vcols_sb[:], in_=vcols_d[:])
            ident_sb = singles.tile([128, 128], f32)
            nc.gpsimd.dma_start(out=ident_sb[:], in_=ident_d[:])
            if has_mask:
                one11 = singles.tile([1, 1], bf16)
                nc.vector.memset(one11, 1.0)
            one_f = singles.tile([1, 1], f32)
            nc.vector.memset(one_f, 1.0)

            import concourse.bass as bass

            def emit_bcast(st):
                # attnbf -> DRAM bounce -> stride-0 partition broadcast
                r = st["r"]
                nc.scalar.dma_start(
                    out=attn_bounce[r : r + 1, :], in_=st["attnbf"][:]
                )
                expb_sb = small.tile([128, S], bf16, name="expb_sb", bufs=3)
                st["expb_sb"] = expb_sb
                brow = attn_bounce[r : r + 1, :]
                bcast_ap = bass.AP(
                    tensor=brow.tensor,
                    offset=brow.offset,
                    ap=[[0, 128], brow.ap[-1]],
                )
                nc.scalar.dma_start(out=expb_sb[:], in_=bcast_ap)

            def emit_ctx_dve(st):
                # ctxT[e%128, chunk] = sum_s encT * attn on DVE (muls +
                # reduces 0..nd-1) and ACT (reduces nd..7), software-pipelined
                nd = st["nd"]
                encT_t, expb_sb = st["encT_t"], st["expb_sb"]
                ct_dve = small.tile([128, nd], mybir.dt.float32, name="ct_dve", tag="ctd", bufs=5)
                ct_act = small.tile(
                    [128, 8 - nd], mybir.dt.float32, name="ct_act", tag="ctc", bufs=5
                )
                st["ct_dve"], st["ct_act"] = ct_dve, ct_act
                scr = [
                    small.tile(
                        [128, S], bf16, name=f"scr{i}", tag=f"scr{i}",
                        bufs=2 if i >= 6 else 1,
                    )
                    for i in range(8)
                ]

                def cmul(c):
                    nc.vector.tensor_mul(scr[c][:], encT_t[c][:], expb_sb[:])

                def cred(c):
                    nc.vector.reduce_sum(
                        out=ct_dve[:, c : c + 1], in_=scr[c][:], axis=mybir.AxisListType.X
                    )

                cmul(0)
                cmul(1)
                for c in range(2, 8):
                    cmul(c)
                    if c - 2 < nd:
                        cred(c - 2)
                st["scr"] = scr

            def emit_ctx_act(st):
                # ACT reductions for chunks nd..7 — emitted after the NEXT
                # row's tanh so they never block it in the ACT FIFO
                nd = st["nd"]
                for c in range(nd, 8):
                    nc.scalar.activation(
                        out=st["scr"][c][:],
                        in_=st["scr"][c][:],
                        func=Ident,
                        accum_out=st["ct_act"][:, c - nd : c - nd + 1],
                    )

            def emit_pe_expT(st):
                # expm row -> partitions via K=1 matmuls (PE)
                st["expT_ps"] = mmps.tile(
                    [128, 8], mybir.dt.float32, tag="mm", name="expT_ps"
                )
                for c in range(8):
                    nc.tensor.matmul(
                        st["expT_ps"][:, c : c + 1],
                        lhsT=st["expm"][0:1, c * 128 : (c + 1) * 128],
                        rhs=one_f[:],
                        start=True,
                        stop=True,
                    )

            def emit_pe_ctx(st):
                r, encN_t = st["r"], st["encN_t"]
                expT_sb = small.tile([128, 8], bf16, name="expT_sb")
                nc.vector.tensor_copy(out=expT_sb[:], in_=st["expT_ps"][:])
                ctx_ps = vecps.tile([1, E], mybir.dt.float32, tag="v", name="ctx_ps")
                for eh in range(2):
                    for c in range(8):
                        nc.tensor.matmul(
                            ctx_ps[0:1, eh * 512 : (eh + 1) * 512],
                            lhsT=expT_sb[:, c : c + 1],
                            rhs=encN_t[c][:, eh * 512 : (eh + 1) * 512],
                            start=(c == 0),
                            stop=(c == 7),
                        )
                outctx = outp.tile([1, E], mybir.dt.float32, tag="outctx", bufs=2)
                nc.scalar.activation(
                    out=outctx[:], in_=ctx_ps[0:1, :], func=Ident, scale=st["recip"]
                )
                nc.gpsimd.dma_start(out=out_d[r : r + 1, 0:E], in_=outctx[:])

            def emit_ctx_out(st):
                # transposes on PE (deferred), copies + DMA out
                r, nd = st["r"], st["nd"]
                na = 8 - nd
                ps_d = vecps.tile([nd, 128], mybir.dt.float32, tag="v", name="ps_d")
                nc.tensor.transpose(ps_d[:], st["ct_dve"][:], ident_sb[:])
                ps_a = vecps.tile([na, 128], mybir.dt.float32, tag="v", name="ps_a")
                nc.tensor.transpose(ps_a[:], st["ct_act"][:], ident_sb[:])
                outc_d = outp.tile([nd, 1, 128], mybir.dt.float32, tag="outcd", bufs=2)
                nc.scalar.copy(out=outc_d[:, 0, :], in_=ps_d[:])
                outc_a = outp.tile([na, 1, 128], mybir.dt.float32, tag="outca", bufs=2)
                nc.scalar.copy(out=outc_a[:, 0, :], in_=ps_a[:])
                nc.gpsimd.dma_start(
                    out=out_d[r : r + 1, 0 : nd * 128].rearrange(
                        "o (j f) -> j o f", j=nd
                    ),
                    in_=outc_d[:],
                )
                nc.gpsimd.dma_start(
                    out=out_d[r : r + 1, nd * 128 : 1024].rearrange(
                        "o (j f) -> j o f", j=na
                    ),
                    in_=outc_a[:],
                )

            # hoisted prefetch: the first two rows' chunk DMAs go out before
            # any result-gated DMA exists, split across both input rings
            prefetched = []
            for rr in range(min(2, rows)):
                tiles = [
                    encT_pool.tile([128, S], bf16, name=f"encc{c}", tag=f"encc{c}")
                    for c in range(8)
                ]
                for c in range(8):
                    eng = nc.scalar if c % 2 == 1 else nc.sync
                    eng.dma_start(out=tiles[c][:], in_=encT_d[rr, :, c, :])
                prefetched.append(tiles)

            pend_out = []
            pend_pe = []
            encN_pending = {}
            for r in range(rows):
                if r < len(prefetched):
                    encT_t = prefetched[r]
                else:
                    encT_t = [
                        encT_pool.tile([128, S], bf16, name=f"encc{c}", tag=f"encc{c}")
                        for c in range(8)
                    ]
                    for c in range(8):
                        nc.sync.dma_start(out=encT_t[c][:], in_=encT_d[r, :, c, :])
                pe_ctx_row = r >= rows - 3
                # prefetch encN (natural layout for the PE-ctx tail rows) two
                # rows early so its DMA issue load spreads out
                rpre = r + 2
                if rows - 3 <= rpre < rows:
                    tiles = [
                        encT_pool.tile([128, E], bf16, name=f"encn{c}", tag=f"encn{c}", bufs=2)
                        for c in range(8)
                    ]
                    for c in range(8):
                        nc.sync.dma_start(
                            out=tiles[c][:], in_=encN_d[rpre - (rows - 3), :, c, :]
                        )
                    encN_pending[rpre] = tiles
                encN_t = encN_pending.pop(r, None)
                if pe_ctx_row and encN_t is None:
                    encN_t = [
                        encT_pool.tile([128, E], bf16, name=f"encn{c}", tag=f"encn{c}", bufs=2)
                        for c in range(8)
                    ]
                    for c in range(8):
                        nc.sync.dma_start(
                            out=encN_t[c][:], in_=encN_d[r - (rows - 3), :, c, :]
                        )
                if has_mask:
                    maskrow = small.tile([1, S], bf16, name="maskrow", bufs=2)
                    nc.sync.dma_start(out=maskrow[:], in_=maskf_d[r : r + 1, :])

                # projT -> tanh -> energyT (bf16), a on partitions
                energyT = []
                for m in range(4):
                    et = energy_pool.tile(
                        [128, S], bf16, name=f"energyT{m}", tag=f"e{m}"
                    )
                    energyT.append(et)
                    for n in range(2):
                        mm = mmps.tile([128, 512], mybir.dt.float32, tag="mm")
                        for k in range(8):
                            nc.tensor.matmul(
                                mm[:],
                                lhsT=w_sb[k][:, m * 128 : (m + 1) * 128],
                                rhs=encT_t[k][:, n * 512 : (n + 1) * 512],
                                start=(k == 0),
                                stop=(k == 7),
                            )
                        nc.scalar.activation(
                            out=et[:, n * 512 : (n + 1) * 512],
                            in_=mm[:],
                            func=Tanh,
                            bias=dpT_sb[:, m, r : r + 1],
                            scale=1.0,
                        )

                # deferred PE work for earlier rows
                if pend_pe:
                    emit_pe_expT(pend_pe[0])
                if len(pend_out) >= 3:
                    emit_ctx_out(pend_out.pop(0))

                # scores flat [1, S]
                scores_ps = vecps.tile([1, S], mybir.dt.float32, tag="v")
                for n in range(2):
                    for m in range(4):
                        nc.tensor.matmul(
                            scores_ps[0:1, n * 512 : (n + 1) * 512],
                            lhsT=vcols_sb[:, m : m + 1],
                            rhs=energyT[m][:, n * 512 : (n + 1) * 512],
                            start=(m == 0),
                            stop=(m == 3) and not has_mask,
                        )
                    if has_mask:
                        nc.tensor.matmul(
                            scores_ps[0:1, n * 512 : (n + 1) * 512],
                            lhsT=one11[:],
                            rhs=maskrow[0:1, n * 512 : (n + 1) * 512],
                            start=False,
                            stop=True,
                        )

                # softmax smalls
                expm = small.tile([1, S], mybir.dt.float32, name="expm", bufs=4)
                srtile = small.tile([1, 8], mybir.dt.float32)
                ssum = srtile[0:1, 0:1]
                recip = srtile[0:1, 1:2]
                nc.scalar.activation(
                    out=expm[:], in_=scores_ps[0:1, :], func=Exp, accum_out=ssum
                )
                nc.vector.reciprocal(out=recip, in_=ssum)

                # attn output (f32) + normalized bf16 copy for the broadcast
                outa = outp.tile([1, S], mybir.dt.float32, tag="outa", bufs=2)
                nc.scalar.activation(out=outa[:], in_=expm[:], func=Ident, scale=recip)
                nc.gpsimd.dma_start(out=out_d[r : r + 1, E : E + S], in_=outa[:])
                attnbf = small.tile([1, S], bf16, name="attnbf")
                nc.scalar.activation(
                    out=attnbf[:], in_=expm[:], func=Ident, scale=recip
                )

                # ACT reductions of the previous row, after this row's
                # softmax outputs so they can't block the chain
                if pend_out:
                    last = pend_out[-1]
                    if not last.get("act_done") and "scr" in last:
                        emit_ctx_act(last)
                        last["act_done"] = True

                if pend_pe:
                    emit_pe_ctx(pend_pe.pop(0))

                st = {
                    "r": r,
                    "encT_t": encT_t,
                    "attnbf": attnbf,
                    "expm": expm,
                    "recip": recip,
                    "encN_t": encN_t,
                    "nd": 6,
                }
                if pe_ctx_row:
                    pend_pe.append(st)
                else:
                    emit_bcast(st)
                    emit_ctx_dve(st)
                    pend_out.append(st)

            for st in pend_out:
                if not st.get("act_done") and "scr" in st:
                    emit_ctx_act(st)
                    st["act_done"] = True
            while pend_out:
                emit_ctx_out(pend_out.pop(0))
            while pend_pe:
                s0 = pend_pe.pop(0)
                emit_pe_expT(s0)
                emit_pe_ctx(s0)

    nc.compile()
    return nc


def _prep_inputs(encoder_outputs, decoder_hidden, src_mask, W_attn, b_attn, v):
    bf16 = ml_dtypes.bfloat16
    b = encoder_outputs.shape[0]
    rows = b // NCORES

    enc_bf = np.asarray(encoder_outputs, dtype=np.float32).astype(bf16)
    # encT_a[b, p, c, s] = enc[b, s, c*128+p]
    encT_a = np.ascontiguousarray(enc_bf.reshape(b, S, 8, 128).transpose(0, 3, 2, 1))
    # natural layout for the last 3 rows of each core: [b, p, c, e] = enc[b, c*128+p, e]
    encN_a = np.ascontiguousarray(enc_bf.reshape(b, 8, 128, E).transpose(0, 2, 1, 3))

    W = np.asarray(W_attn, dtype=np.float32)
    w_a = np.ascontiguousarray(W[:E].astype(bf16).reshape(8, 128, A).transpose(1, 0, 2))
    W2 = W[E:]
    b_np = np.asarray(b_attn, dtype=np.float32)
    vcols = np.ascontiguousarray(
        np.asarray(v, dtype=np.float32).astype(bf16).reshape(4, 128).T
    )
    ident = np.eye(128, dtype=np.float32)

    dh = np.asarray(decoder_hidden, dtype=np.float32)
    mask_np = np.asarray(src_mask)
    has_mask = bool((mask_np == 0).any())
    # additive log-mask: 0 where kept, -1e4 where masked (exp -> exactly 0)
    maskf = np.where(mask_np != 0, 0.0, -1e4).astype(bf16)

    in_maps = []
    for i in range(NCORES):
        sl = slice(i * rows, (i + 1) * rows)
        dh_sh = dh[sl]  # [rows, DH]
        # dec_proj + b on host (exact f32), transposed to [128, 4, rows]
        dp = (dh_sh @ W2 + b_np).astype(np.float32)  # [rows, A]
        dpT_a = np.ascontiguousarray(
            dp.T.reshape(4, 128, rows).transpose(1, 0, 2)
        )
        m = {
            "encT": encT_a[sl],
            "encN": encN_a[i * rows + rows - 3 : (i + 1) * rows],
            "w": w_a,
            "dpT": dpT_a,
            "vcols": vcols,
            "ident": ident,
        }
        if has_mask:
            m["maskf"] = np.ascontiguousarray(maskf[sl])
        in_maps.append(m)
    return in_maps, rows, has_mask


def kernel(encoder_outputs, decoder_hidden, src_mask, W_attn, b_attn, v):
    global LAST_EXEC_NS
    from concourse.bass_utils import run_bass_kernel_spmd

    in_maps, rows, has_mask = _prep_inputs(
        encoder_outputs, decoder_hidden, src_mask, W_attn, b_attn, v
    )

    key = (rows, has_mask)
    if key not in _NC_CACHE:
        _NC_CACHE[key] = _build_nc(rows, has_mask)
    nc = _NC_CACHE[key]

    trace = os.environ.get("KERNEL_TRACE", "0") == "1"
    res = run_bass_kernel_spmd(nc, in_maps, core_ids=list(range(NCORES)), trace=trace)
    LAST_EXEC_NS = res.exec_time_ns

    ctx = np.concatenate([r["out"][:, :E] for r in res.results], axis=0)
    attn = np.concatenate([r["out"][:, E:] for r in res.results], axis=0)
    return ctx.astype(np.float32), attn.astype(np.float32)
